# revision 31
# baseline (speedup 1.0000x reference)
"""3-layer GAT on trn2, 8 NeuronCores, edge-parallel with dst-range sharding.

Per core c (owning dst nodes [c*2500, (c+1)*2500)), edges bucketed by dst into
20 windows of 125 nodes, padded per-window to a multiple of 128 (window sizes
maxed over cores so the SPMD program is identical everywhere).

v2 design (vs baseline): fp16 gather table with interleaved [h_k|1]xheads
rows (the 1-columns make the scatter matmul emit softmax denominators for
free) and per-edge `as` stored as fp32 inside the fp16 row via a bitcast
view; as/ad folded into the dense matmul via host-precomputed W@a columns;
the transposed one-hot (ohT) host-precomputed and persistent in SBUF; the
per-window score pipeline is batched (one TT add, one fused leaky
scalar_tensor_tensor, one Act Exp that also expands p per-head to 65 cols via
a stride-0 view, one packed fp16 TT for the p-weighting at DVE 2x mode); per
chunk only a 4x-mode fp16 one-hot build on DVE plus two PE matmuls (ad
gather + scatter). Epilogue divides via one broadcast TT; bias+relu run on
the Act engine in transposed layout where bias is per-partition. The
dma_gather row must be a multiple of 256B, hence row padding to 384/128
elems.
"""
import os, sys
for _p in ('/opt/trn_rl_repo', '/root/.axon_site/_ro/trn_rl_repo'):
    if os.path.isdir(_p) and _p not in sys.path:
        sys.path.insert(0, _p)

import numpy as np

import concourse.bacc as bacc
import concourse.tile as tile
from concourse import bass, mybir
from concourse import bass_utils

N = 20000
E = 320000
HID = 64
HEADS = 4
OUT_CH = 64
NEG = 0.2
C = 8
SHARD = N // C          # 2500
WIN = 125               # dst nodes per window
NW = SHARD // WIN       # 20
P = 128

# fin, fout, heads, scat (=65*heads, scatter cols), row (gather row elems,
# 256B-multiple for dma_gather), asf (fp32 col of `as` in the row's f32
# bitcast view), wcols (dense-matmul rhs cols: scat | as | ad), cont (row
# content cols actually written/read; the rest is dma_gather row padding)
LAYERS = [
    dict(fin=64,  fout=256, heads=4, scat=260, row=384, asf=130, wcols=268,
         cont=268),
    dict(fin=256, fout=256, heads=4, scat=260, row=384, asf=130, wcols=268,
         cont=268),
    dict(fin=256, fout=64,  heads=1, scat=65,  row=128, asf=33,  wcols=67,
         cont=68),
]

AX = mybir.AxisListType
ALU = mybir.AluOpType
ACTF = mybir.ActivationFunctionType
F32 = mybir.dt.float32
F16 = mybir.dt.float16
I16 = mybir.dt.int16
NPDT = np.float16


def _host_prep(edge_index):
    """Per-core gather idx / dstloc / transposed-onehot arrays + window sizes."""
    src = np.asarray(edge_index[0], dtype=np.int64)
    dst = np.asarray(edge_index[1], dtype=np.int64)
    per_core = []   # (srcs, dstloc) per (core, window)
    counts = np.zeros((C, NW), dtype=np.int64)
    for c in range(C):
        m = (dst >= c * SHARD) & (dst < (c + 1) * SHARD)
        es, ed = src[m], dst[m] - c * SHARD
        order = np.argsort(ed, kind='stable')
        es, ed = es[order], ed[order]
        w = ed // WIN
        wins = []
        for wi in range(NW):
            sel = w == wi
            ws, wd = es[sel], ed[sel] - wi * WIN
            # sort the window's edges by src so the gather walks HBM in
            # ascending address order (scatter is one-hot-based, so edge
            # order within a window is free)
            o = np.argsort(ws, kind='stable')
            wins.append((ws[o], wd[o]))
            counts[c, wi] = sel.sum()
        per_core.append(wins)
    kws = (np.ceil(counts.max(axis=0) / P).astype(np.int64) * P)
    kws = np.maximum(kws, P)
    tot = int(kws.sum())
    idx_all, dl_all, oht_all = [], [], []
    for c in range(C):
        idx_mat = np.zeros((16, tot // 16), dtype=np.int16)
        dl_mat = np.full((P, tot // P), float(WIN), dtype=np.float32)
        oht = np.zeros((WIN, tot), dtype=NPDT)
        icol = ccol = ecol = 0
        for wi in range(NW):
            kw = int(kws[wi])
            es, dl = per_core[c][wi]
            n = len(es)
            sp = np.zeros(kw, dtype=np.int16)
            dp = np.full(kw, float(WIN), dtype=np.float32)
            sp[:n] = es.astype(np.int16)
            dp[:n] = dl.astype(np.float32)
            idx_mat[:, icol:icol + kw // 16] = sp.reshape(-1, 16).T
            dl_mat[:, ccol:ccol + kw // P] = dp.reshape(-1, P).T
            oht[dl.astype(np.int64), ecol + np.arange(n)] = NPDT(1.0)
            icol += kw // 16
            ccol += kw // P
            ecol += kw
        idx_all.append(np.tile(idx_mat, (8, 1)))
        dl_all.append(dl_mat)
        oht_all.append(oht)
    return tuple(int(k) for k in kws), idx_all, dl_all, oht_all


def build(kws, timing_reps=0):
    """Builds the SPMD bass module. kws: per-window padded edge counts."""
    tot = sum(kws)
    tws = [k // P for k in kws]
    nc = bacc.Bacc("TRN2", target_bir_lowering=False, debug=False, num_devices=C)

    # ---- DRAM I/O ----
    d_xT = nc.dram_tensor("xT_own", [HID, SHARD], F16, kind="ExternalInput")
    d_W = [nc.dram_tensor(f"Wx{l+1}", [LAYERS[l]['fin'], LAYERS[l]['wcols']],
                          F16, kind="ExternalInput") for l in range(3)]
    d_bc = nc.dram_tensor("bc12", [P, 4], F32, kind="ExternalInput")
    d_b3 = nc.dram_tensor("b3r", [P, OUT_CH], F32, kind="ExternalInput")
    d_ident = nc.dram_tensor("ident", [P, P], F32, kind="ExternalInput")
    d_idx = nc.dram_tensor("gat_idx", [P, tot // 16], I16, kind="ExternalInput")
    d_dl16 = nc.dram_tensor("dstloc16", [P, tot // P], F16,
                            kind="ExternalInput")
    twmax = max(tws)
    d_iorep = nc.dram_tensor("iota_rep", [P, twmax * WIN], F16,
                             kind="ExternalInput")
    d_oht = nc.dram_tensor("ohT", [WIN, tot], F16, kind="ExternalInput")
    d_out = nc.dram_tensor("out", [SHARD, OUT_CH], F32, kind="ExternalOutput")
    if timing_reps:
        d_tok = nc.dram_tensor("tok", [1, 32], F32, kind="ExternalInput")
        d_toko = nc.dram_tensor("tok_out", [1, 32], F32, kind="ExternalOutput")

    tabs = []
    for l, cfg in enumerate(LAYERS):
        s = nc.dram_tensor(f"tab{l+1}s", [SHARD, cfg['row']], F16)
        f = nc.dram_tensor(f"tab{l+1}f", [N, cfg['row']], F16,
                           addr_space="Shared")
        tabs.append((s, f))

    with tile.TileContext(nc) as tc:
        with tc.tile_pool(name="const", bufs=1) as cp, \
             tc.tile_pool(name="rowp", bufs=2) as rowp, \
             tc.tile_pool(name="gp", bufs=2) as gp, \
             tc.tile_pool(name="sp", bufs=3) as sp, \
             tc.tile_pool(name="rp", bufs=2) as rp, \
             tc.tile_pool(name="op", bufs=2) as op_, \
             tc.tile_pool(name="ps", bufs=1, space="PSUM") as pp:

            # ---- persistent SBUF ----
            ident = cp.tile([P, P], F32)
            nc.sync.dma_start(ident[:], d_ident[:, :])
            idx_sb = cp.tile([P, tot // 16], I16)
            nc.sync.dma_start(idx_sb[:], d_idx[:, :])
            dl16_sb = cp.tile([P, tot // P], F16)
            nc.sync.dma_start(dl16_sb[:], d_dl16[:, :])
            iorep = cp.tile([P, twmax * WIN], F16)
            nc.sync.dma_start(iorep[:], d_iorep[:, :])
            oht_sb = cp.tile([WIN, tot], F16)
            nc.sync.dma_start(oht_sb[:], d_oht[:, :])
            xT = cp.tile([HID, SHARD], F16)
            nc.sync.dma_start(xT[:], d_xT[:, :])
            bc12 = cp.tile([P, 4], F32)
            nc.sync.dma_start(bc12[:], d_bc[:, :])
            b3r = cp.tile([P, OUT_CH], F32)
            nc.sync.dma_start(b3r[:], d_b3[:, :])
            Wt = []
            for l, cfg in enumerate(LAYERS):
                fin, wcols = cfg['fin'], cfg['wcols']
                chunks = []
                for kc in range(0, fin, P):
                    ke = min(kc + P, fin)
                    t = cp.tile([ke - kc, wcols], F16, tag=f"W{l}_{kc}")
                    nc.sync.dma_start(t[:], d_W[l][kc:ke, :])
                    chunks.append(t)
                Wt.append(chunks)
            actT = {1: [cp.tile([P, SHARD], F16, tag=f"actT1_{j}",
                                name=f"actT1_{j}") for j in range(2)],
                    2: [cp.tile([P, SHARD], F16, tag=f"actT2_{j}",
                                name=f"actT2_{j}") for j in range(2)]}
            ado = [cp.tile([P, NW * LAYERS[l]['heads']], F16, tag=f"ado{l}",
                           name=f"ado{l}") for l in range(3)]

            def dense_window(l, w, actT_in):
                cfg = LAYERS[l]
                heads, scat, row, wcols = (cfg['heads'], cfg['scat'],
                                           cfg['row'], cfg['wcols'])
                asf = cfg['asf']
                tab_s, tab_f = tabs[l]
                nchunks = len(Wt[l])
                ph = pp.tile([WIN, wcols], F32, tag="ph", bufs=2)
                for kc in range(nchunks):
                    nc.tensor.matmul(
                        ph[:, :], lhsT=actT_in[kc][:, w * WIN:(w + 1) * WIN],
                        rhs=Wt[l][kc][:], start=(kc == 0),
                        stop=(kc == nchunks - 1))
                row_t = rowp.tile([P, row], F16, tag="row")
                nc.vector.tensor_copy(row_t[:WIN, 0:scat], ph[:, 0:scat])
                ones_v = row_t[:WIN, 0:scat].rearrange(
                    "p (h x) -> p h x", x=65)[:, :, 64:65]
                nc.vector.memset(ones_v, 1.0)
                row_f = row_t[:].bitcast(F32)
                nc.vector.tensor_copy(row_f[:WIN, asf:asf + heads],
                                      ph[:, scat:scat + heads])
                nc.vector.tensor_copy(ado[l][:WIN, w * heads:(w + 1) * heads],
                                      ph[:, scat + heads:scat + 2 * heads])
                cont = cfg['cont']
                if timing_reps:
                    dst = tab_f[:, :].rearrange("(s n) e -> n s e", s=C)[
                        w * WIN:(w + 1) * WIN, :, 0:cont]
                    src = row_t[:WIN, 0:cont].rearrange(
                        "p (o e) -> p o e", o=1).to_broadcast([WIN, C, cont])
                    nc.sync.dma_start(dst, src)
                else:
                    nc.sync.dma_start(tab_s[w * WIN:(w + 1) * WIN, 0:cont],
                                      row_t[:WIN, 0:cont])

            def allgather(l):
                if not timing_reps:
                    tab_s, tab_f = tabs[l]
                    nc.gpsimd.collective_compute(
                        "AllGather", ALU.bypass,
                        replica_groups=[list(range(C))],
                        ins=[tab_s[:, :]], outs=[tab_f[:, :]])

            def oh_window(l, w, ccol):
                """One batched one-hot build for all of window w's chunks."""
                tw = tws[w]
                eng = (nc.gpsimd if os.environ.get("GAT_OH", "dve") == "pool"
                       else nc.vector)
                oh_all = sp.tile([P, tw * WIN], F16, tag="ohall", bufs=2)
                if 'nooh' in abl:
                    nc.vector.memset(oh_all[0:1, 0:2], 0.0)
                else:
                    eng.tensor_tensor(
                        oh_all[:].rearrange("p (t x) -> p t x", x=WIN),
                        iorep[:, 0:tw * WIN].rearrange(
                            "p (t x) -> p t x", x=WIN),
                        dl16_sb[:, ccol:ccol + tw].rearrange(
                            "p (t o) -> p t o", o=1).to_broadcast(
                                [P, tw, WIN]),
                        op=ALU.is_equal)
                return oh_all

            abl = set(os.environ.get("GAT_ABL", "").split(","))

            def edge_front(l, w, icol, ccol, ecol):
                """gather + pads + onehot + scores + weighting for window w."""
                cfg = LAYERS[l]
                heads, scat, row = cfg['heads'], cfg['scat'], cfg['row']
                asf = cfg['asf']
                tab_f = tabs[l][1]
                kw, tw = kws[w], tws[w]
                gw = gp.tile([P, tw * row], F16, tag="gw")
                gw3 = gw[:].rearrange("p (t e) -> p t e", e=row)
                if 'nogather' not in abl:
                    nc.gpsimd.dma_gather(
                        gw3, tab_f[:, :], idx_sb[:, icol:icol + kw // 16],
                        kw, kw, row, single_packet=False)
                else:
                    nc.vector.memset(gw[0:1, 0:2], 0.0)
                oh_all = oh_window(l, w, ccol)
                ad_w = ado[l][:WIN, w * heads:(w + 1) * heads]
                pads = pp.tile([P, tw * heads], F32, tag="pads", bufs=2)
                if 'nope' not in abl:
                    for t in range(tw):
                        nc.tensor.matmul(
                            pads[:, t * heads:(t + 1) * heads],
                            lhsT=oht_sb[:, ecol + t * P:ecol + (t + 1) * P],
                            rhs=ad_w, start=True, stop=True)
                else:
                    nc.vector.memset(pads[0:1, 0:2], 0.0)
                gwf = gw[:].bitcast(F32).rearrange("p (t e) -> p t e",
                                                   e=row // 2)
                st = sp.tile([P, tw * heads], F32, tag="st")
                lt = sp.tile([P, tw * heads], F32, tag="lt")
                if 'nost' not in abl:
                    nc.vector.tensor_tensor(
                        st[:].rearrange("p (t h) -> p t h", h=heads),
                        gwf[:, :, asf:asf + heads], pads[:].rearrange(
                            "p (t h) -> p t h", h=heads), op=ALU.add)
                    nc.vector.scalar_tensor_tensor(lt[:], st[:], NEG, st[:],
                                                   op0=ALU.mult, op1=ALU.max)
                else:
                    nc.vector.memset(st[0:1, 0:2], 0.0)
                    nc.vector.memset(lt[0:1, 0:2], 0.0)
                p_rep = rp.tile([P, tw * scat], F16, tag="prep")
                lt_b = lt[:].rearrange("p (t h) -> p t h", h=heads)
                if 'noexp' not in abl:
                    nc.scalar.activation(
                        p_rep[:].rearrange("p (t h x) -> p t h x",
                                           h=heads, x=65),
                        lt_b.broadcast_to((P, tw, heads, 65)), ACTF.Exp)
                else:
                    nc.vector.memset(p_rep[0:1, 0:2], 0.0)
                rhs_all = rp.tile([P, tw * scat], F16, tag="rhs")
                if 'noweight' not in abl:
                    nc.vector.tensor_tensor(
                        rhs_all[:].rearrange("p (t e) -> p t e", e=scat),
                        gw3[:, :, 0:scat], p_rep[:].rearrange(
                            "p (t e) -> p t e", e=scat), op=ALU.mult)
                else:
                    nc.vector.memset(rhs_all[0:1, 0:2], 0.0)
                return oh_all, rhs_all

            def edge_back(l, w, fr, actT_next):
                """scatter + epilogue for window w."""
                cfg = LAYERS[l]
                heads, scat = cfg['heads'], cfg['scat']
                tw = tws[w]
                oh_all, rhs_all = fr
                psw = pp.tile([WIN, scat], F32, tag="psw", bufs=2)
                if 'nope' not in abl:
                    for t in range(tw):
                        nc.tensor.matmul(
                            psw[:], lhsT=oh_all[:, t * WIN:(t + 1) * WIN],
                            rhs=rhs_all[:, t * scat:(t + 1) * scat],
                            start=(t == 0), stop=(t == tw - 1))
                else:
                    nc.vector.memset(psw[0:1, 0:2], 0.0)
                ps3 = psw[:].rearrange("p (h x) -> p h x", x=65)
                dn = sp.tile([WIN, heads], F32, tag="dn")
                nc.vector.tensor_scalar(dn[:], ps3[:, :, 64:65], 1e-16, None,
                                        op0=ALU.add)
                rec = sp.tile([WIN, heads], F32, tag="rec")
                nc.vector.reciprocal(rec[:], dn[:])
                out_t = op_.tile([WIN, heads * 64], F32, tag="out_t")
                nc.vector.tensor_tensor(
                    out_t[:].rearrange("p (h x) -> p h x", x=64),
                    ps3[:, :, 0:64], rec[:].broadcast_to((WIN, heads, 64)),
                    op=ALU.mult)
                if l < 2:
                    for j in range(2):
                        pt = pp.tile([P, WIN], F32, tag="pt", bufs=2)
                        if 'nope' not in abl:
                            nc.tensor.transpose(pt[:],
                                                out_t[:, j * P:(j + 1) * P],
                                                ident[:WIN, :WIN])
                        else:
                            nc.vector.memset(pt[0:1, 0:2], 0.0)
                        nc.scalar.activation(
                            actT_next[j][:, w * WIN:(w + 1) * WIN], pt[:],
                            ACTF.Relu, bias=bc12[:, 2 * l + j:2 * l + j + 1])
                else:
                    orow = op_.tile([WIN, OUT_CH], F32, tag="orow")
                    nc.vector.tensor_tensor(orow[:], out_t[:], b3r[:WIN, :],
                                            op=ALU.add)
                    nc.sync.dma_start(d_out[w * WIN:(w + 1) * WIN, :],
                                      orow[:])

            def edge_dense_phase(l, actT_next, next_in):
                """edge(l) with a 1-window software-pipeline skew: window
                w+1's front (gather/pads/onehot/scores/weighting) is issued
                before window w's back (scatter/epilogue), so no engine queue
                blocks the next window behind the current window's tail.
                dense(l+1) is interleaved per window so the next layer's
                table+AG is ready as edge(l) drains."""
                offs, icol, ccol, ecol = [], 0, 0, 0
                for w in range(NW):
                    offs.append((icol, ccol, ecol))
                    icol += kws[w] // 16
                    ccol += kws[w] // P
                    ecol += kws[w]
                fr_prev = None
                for w in range(NW):
                    fr = edge_front(l, w, *offs[w])
                    if fr_prev is not None:
                        edge_back(l, w - 1, fr_prev, actT_next)
                        if next_in is not None:
                            dense_window(l + 1, w - 1, next_in)
                    fr_prev = fr
                edge_back(l, NW - 1, fr_prev, actT_next)
                if next_in is not None:
                    dense_window(l + 1, NW - 1, next_in)
                    allgather(l + 1)

            def body():
                stages = int(os.environ.get("GAT_STAGES", "9"))
                for w in range(NW):
                    dense_window(0, w, [xT])
                allgather(0)
                if stages >= 3:
                    edge_dense_phase(0, actT[1], actT[1] if stages >= 4 else None)
                if stages >= 5:
                    edge_dense_phase(1, actT[2], actT[2] if stages >= 6 else None)
                if stages >= 7:
                    edge_dense_phase(2, None, None)
                if stages < 7:
                    z = op_.tile([WIN, OUT_CH], F32, tag="z", name="z")
                    nc.vector.memset(z[:], 0.0)
                    for w in range(NW):
                        nc.sync.dma_start(d_out[w * WIN:(w + 1) * WIN, :], z[:])

            if timing_reps:
                tk = cp.tile([1, 32], F32)
                nc.sync.dma_start(tk[:], d_tok[:, :])
                if timing_reps == 1:
                    body()
                else:
                    with tc.For_i(0, timing_reps, 1):
                        body()
                nc.sync.dma_start(d_toko[:, :], tk[:])
            else:
                body()

    nc.compile()
    return nc


def _host_inputs(x, edge_index, W1, a1s, a1d, b1, W2, a2s, a2d, b2, W3, a3s, a3d, b3):
    kws, idx_all, dl_all, oht_all = _host_prep(edge_index)
    x = np.asarray(x, dtype=np.float32)
    Ws = [np.asarray(W1, np.float32), np.asarray(W2, np.float32),
          np.asarray(W3, np.float32)]
    As = [np.asarray(a1s, np.float32), np.asarray(a2s, np.float32),
          np.asarray(a3s, np.float32)]
    Ad = [np.asarray(a1d, np.float32), np.asarray(a2d, np.float32),
          np.asarray(a3d, np.float32)]
    bs = [np.asarray(b1, np.float32), np.asarray(b2, np.float32),
          np.asarray(b3, np.float32)]
    shared = {}
    for l, cfg in enumerate(LAYERS):
        fin, heads, scat, wcols = (cfg['fin'], cfg['heads'], cfg['scat'],
                                   cfg['wcols'])
        W, a_s, a_d = Ws[l], As[l], Ad[l]
        Wext = np.zeros((fin, wcols), np.float32)
        for k in range(heads):
            Wk = W[:, 64 * k:64 * (k + 1)]
            Wext[:, 65 * k:65 * k + 64] = Wk
            Wext[:, scat + k] = Wk @ a_s[k]
            Wext[:, scat + heads + k] = Wk @ a_d[k]
        shared[f"Wx{l+1}"] = Wext.astype(NPDT)
    bc = np.zeros((P, 4), np.float32)
    bc[:, 0] = bs[0][0:P]
    bc[:, 1] = bs[0][P:2 * P]
    bc[:, 2] = bs[1][0:P]
    bc[:, 3] = bs[1][P:2 * P]
    shared["bc12"] = bc
    shared["b3r"] = np.tile(bs[2].reshape(1, OUT_CH), (P, 1))
    shared["ident"] = np.eye(P, dtype=np.float32)
    twmax = max(k // P for k in kws)
    shared["iota_rep"] = np.tile(
        np.arange(WIN, dtype=np.float32).reshape(1, WIN),
        (P, twmax)).astype(NPDT)
    in_maps = []
    for c in range(C):
        m = dict(shared)
        m["xT_own"] = np.ascontiguousarray(
            x[c * SHARD:(c + 1) * SHARD].T).astype(NPDT)
        m["gat_idx"] = idx_all[c]
        m["dstloc16"] = dl_all[c].astype(NPDT)
        m["ohT"] = oht_all[c]
        in_maps.append(m)
    return kws, in_maps


_CACHE = {}


def kernel(**inputs) -> np.ndarray:
    kws, in_maps = _host_inputs(**inputs)
    if kws not in _CACHE:
        _CACHE[kws] = build(kws)
    nc = _CACHE[kws]
    last = None
    for _attempt in range(2):
        try:
            res = bass_utils.run_bass_kernel_spmd(
                nc, in_maps, core_ids=list(range(C)), trace=False)
            return np.concatenate(
                [res.results[c]["out"] for c in range(C)], axis=0)
        except Exception as e:  # rare transient device-mesh hiccups: retry once
            last = e
    raise last


# revision 33
# speedup vs baseline: 1.0583x; 1.0583x over previous
"""3-layer GAT on trn2, 8 NeuronCores, edge-parallel with dst-range sharding.

Per core c (owning dst nodes [c*2500, (c+1)*2500)), edges bucketed by dst into
20 windows of 125 nodes, padded per-window to a multiple of 128 (window sizes
maxed over cores so the SPMD program is identical everywhere).

v2 design (vs baseline): fp16 gather table with interleaved [h_k|1]xheads
rows (the 1-columns make the scatter matmul emit softmax denominators for
free) and per-edge `as` stored as fp32 inside the fp16 row via a bitcast
view; as/ad folded into the dense matmul via host-precomputed W@a columns;
the transposed one-hot (ohT) host-precomputed and persistent in SBUF; the
per-window score pipeline is batched (one TT add, one fused leaky
scalar_tensor_tensor, one Act Exp that also expands p per-head to 65 cols via
a stride-0 view, one packed fp16 TT for the p-weighting at DVE 2x mode); per
chunk only a 4x-mode fp16 one-hot build on DVE plus two PE matmuls (ad
gather + scatter). Epilogue divides via one broadcast TT; bias+relu run on
the Act engine in transposed layout where bias is per-partition. The
dma_gather row must be a multiple of 256B, hence row padding to 384/128
elems.
"""
import os, sys
for _p in ('/opt/trn_rl_repo', '/root/.axon_site/_ro/trn_rl_repo'):
    if os.path.isdir(_p) and _p not in sys.path:
        sys.path.insert(0, _p)

import numpy as np

import concourse.bacc as bacc
import concourse.tile as tile
from concourse import bass, mybir
from concourse import bass_utils

N = 20000
E = 320000
HID = 64
HEADS = 4
OUT_CH = 64
NEG = 0.2
C = 8
SHARD = N // C          # 2500
WIN = 125               # dst nodes per window
NW = SHARD // WIN       # 20
P = 128

# fin, fout, heads, scat (=65*heads, scatter cols), row (gather row elems,
# 256B-multiple for dma_gather), asf (fp32 col of `as` in the row's f32
# bitcast view), wcols (dense-matmul rhs cols: scat | as | ad), cont (row
# content cols actually written/read; the rest is dma_gather row padding)
LAYERS = [
    dict(fin=64,  fout=256, heads=4, scat=260, row=384, asf=130, wcols=268,
         cont=268),
    dict(fin=256, fout=256, heads=4, scat=260, row=384, asf=130, wcols=268,
         cont=268),
    dict(fin=256, fout=64,  heads=1, scat=65,  row=128, asf=33,  wcols=67,
         cont=68),
]

AX = mybir.AxisListType
ALU = mybir.AluOpType
ACTF = mybir.ActivationFunctionType
F32 = mybir.dt.float32
F16 = mybir.dt.float16
I16 = mybir.dt.int16
NPDT = np.float16


def _host_prep(edge_index):
    """Per-core gather idx / dstloc / transposed-onehot arrays + window sizes."""
    src = np.asarray(edge_index[0], dtype=np.int64)
    dst = np.asarray(edge_index[1], dtype=np.int64)
    per_core = []   # (srcs, dstloc) per (core, window)
    counts = np.zeros((C, NW), dtype=np.int64)
    for c in range(C):
        m = (dst >= c * SHARD) & (dst < (c + 1) * SHARD)
        es, ed = src[m], dst[m] - c * SHARD
        order = np.argsort(ed, kind='stable')
        es, ed = es[order], ed[order]
        w = ed // WIN
        wins = []
        for wi in range(NW):
            sel = w == wi
            ws, wd = es[sel], ed[sel] - wi * WIN
            # sort the window's edges by src so the gather walks HBM in
            # ascending address order (scatter is one-hot-based, so edge
            # order within a window is free)
            o = np.argsort(ws, kind='stable')
            wins.append((ws[o], wd[o]))
            counts[c, wi] = sel.sum()
        per_core.append(wins)
    kws = (np.ceil(counts.max(axis=0) / P).astype(np.int64) * P)
    kws = np.maximum(kws, P)
    tot = int(kws.sum())
    idx_all, dl_all, oht_all = [], [], []
    for c in range(C):
        idx_mat = np.zeros((16, tot // 16), dtype=np.int16)
        dl_mat = np.full((P, tot // P), float(WIN), dtype=np.float32)
        oht = np.zeros((WIN, tot), dtype=NPDT)
        icol = ccol = ecol = 0
        for wi in range(NW):
            kw = int(kws[wi])
            es, dl = per_core[c][wi]
            n = len(es)
            sp = np.zeros(kw, dtype=np.int16)
            dp = np.full(kw, float(WIN), dtype=np.float32)
            sp[:n] = es.astype(np.int16)
            dp[:n] = dl.astype(np.float32)
            idx_mat[:, icol:icol + kw // 16] = sp.reshape(-1, 16).T
            dl_mat[:, ccol:ccol + kw // P] = dp.reshape(-1, P).T
            oht[dl.astype(np.int64), ecol + np.arange(n)] = NPDT(1.0)
            icol += kw // 16
            ccol += kw // P
            ecol += kw
        idx_all.append(np.tile(idx_mat, (8, 1)))
        dl_all.append(dl_mat)
        oht_all.append(oht)
    return tuple(int(k) for k in kws), idx_all, dl_all, oht_all


def build(kws, timing_reps=0):
    """Builds the SPMD bass module. kws: per-window padded edge counts."""
    tot = sum(kws)
    tws = [k // P for k in kws]
    nc = bacc.Bacc("TRN2", target_bir_lowering=False, debug=False,
                   num_devices=C, num_swdge_queues=4)

    # ---- DRAM I/O ----
    d_xT = nc.dram_tensor("xT_own", [HID, SHARD], F16, kind="ExternalInput")
    d_W = [nc.dram_tensor(f"Wx{l+1}", [LAYERS[l]['fin'], LAYERS[l]['wcols']],
                          F16, kind="ExternalInput") for l in range(3)]
    d_bc = nc.dram_tensor("bc12", [P, 4], F32, kind="ExternalInput")
    d_b3 = nc.dram_tensor("b3r", [P, OUT_CH], F32, kind="ExternalInput")
    d_ident = nc.dram_tensor("ident", [P, P], F32, kind="ExternalInput")
    d_idx = nc.dram_tensor("gat_idx", [P, tot // 16], I16, kind="ExternalInput")
    d_dl16 = nc.dram_tensor("dstloc16", [P, tot // P], F16,
                            kind="ExternalInput")
    twmax = max(tws)
    d_iorep = nc.dram_tensor("iota_rep", [P, twmax * WIN], F16,
                             kind="ExternalInput")
    d_oht = nc.dram_tensor("ohT", [WIN, tot], F16, kind="ExternalInput")
    d_out = nc.dram_tensor("out", [SHARD, OUT_CH], F32, kind="ExternalOutput")
    if timing_reps:
        d_tok = nc.dram_tensor("tok", [1, 32], F32, kind="ExternalInput")
        d_toko = nc.dram_tensor("tok_out", [1, 32], F32, kind="ExternalOutput")

    tabs = []
    for l, cfg in enumerate(LAYERS):
        s = nc.dram_tensor(f"tab{l+1}s", [SHARD, cfg['row']], F16)
        f = nc.dram_tensor(f"tab{l+1}f", [N, cfg['row']], F16,
                           addr_space="Shared")
        tabs.append((s, f))

    with tile.TileContext(nc) as tc:
        with tc.tile_pool(name="const", bufs=1) as cp, \
             tc.tile_pool(name="rowp", bufs=2) as rowp, \
             tc.tile_pool(name="gp", bufs=2) as gp, \
             tc.tile_pool(name="sp", bufs=3) as sp, \
             tc.tile_pool(name="rp", bufs=2) as rp, \
             tc.tile_pool(name="op", bufs=2) as op_, \
             tc.tile_pool(name="ps", bufs=1, space="PSUM") as pp:

            # ---- persistent SBUF ----
            ident = cp.tile([P, P], F32)
            nc.sync.dma_start(ident[:], d_ident[:, :])
            idx_sb = cp.tile([P, tot // 16], I16)
            nc.sync.dma_start(idx_sb[:], d_idx[:, :])
            dl16_sb = cp.tile([P, tot // P], F16)
            nc.sync.dma_start(dl16_sb[:], d_dl16[:, :])
            iorep = cp.tile([P, twmax * WIN], F16)
            nc.sync.dma_start(iorep[:], d_iorep[:, :])
            oht_sb = cp.tile([WIN, tot], F16)
            nc.sync.dma_start(oht_sb[:], d_oht[:, :])
            xT = cp.tile([HID, SHARD], F16)
            nc.sync.dma_start(xT[:], d_xT[:, :])
            bc12 = cp.tile([P, 4], F32)
            nc.sync.dma_start(bc12[:], d_bc[:, :])
            b3r = cp.tile([P, OUT_CH], F32)
            nc.sync.dma_start(b3r[:], d_b3[:, :])
            Wt = []
            for l, cfg in enumerate(LAYERS):
                fin, wcols = cfg['fin'], cfg['wcols']
                chunks = []
                for kc in range(0, fin, P):
                    ke = min(kc + P, fin)
                    t = cp.tile([ke - kc, wcols], F16, tag=f"W{l}_{kc}")
                    nc.sync.dma_start(t[:], d_W[l][kc:ke, :])
                    chunks.append(t)
                Wt.append(chunks)
            actT = {1: [cp.tile([P, SHARD], F16, tag=f"actT1_{j}",
                                name=f"actT1_{j}") for j in range(2)],
                    2: [cp.tile([P, SHARD], F16, tag=f"actT2_{j}",
                                name=f"actT2_{j}") for j in range(2)]}
            ado = [cp.tile([P, NW * LAYERS[l]['heads']], F16, tag=f"ado{l}",
                           name=f"ado{l}") for l in range(3)]

            def dense_window(l, w, actT_in):
                cfg = LAYERS[l]
                heads, scat, row, wcols = (cfg['heads'], cfg['scat'],
                                           cfg['row'], cfg['wcols'])
                asf = cfg['asf']
                tab_s, tab_f = tabs[l]
                nchunks = len(Wt[l])
                ph = pp.tile([WIN, wcols], F32, tag="ph", bufs=2)
                for kc in range(nchunks):
                    nc.tensor.matmul(
                        ph[:, :], lhsT=actT_in[kc][:, w * WIN:(w + 1) * WIN],
                        rhs=Wt[l][kc][:], start=(kc == 0),
                        stop=(kc == nchunks - 1))
                row_t = rowp.tile([P, row], F16, tag="row")
                nc.vector.tensor_copy(row_t[:WIN, 0:scat], ph[:, 0:scat])
                ones_v = row_t[:WIN, 0:scat].rearrange(
                    "p (h x) -> p h x", x=65)[:, :, 64:65]
                nc.vector.memset(ones_v, 1.0)
                row_f = row_t[:].bitcast(F32)
                nc.vector.tensor_copy(row_f[:WIN, asf:asf + heads],
                                      ph[:, scat:scat + heads])
                nc.vector.tensor_copy(ado[l][:WIN, w * heads:(w + 1) * heads],
                                      ph[:, scat + heads:scat + 2 * heads])
                cont = cfg['cont']
                if timing_reps:
                    dst = tab_f[:, :].rearrange("(s n) e -> n s e", s=C)[
                        w * WIN:(w + 1) * WIN, :, 0:cont]
                    src = row_t[:WIN, 0:cont].rearrange(
                        "p (o e) -> p o e", o=1).to_broadcast([WIN, C, cont])
                    nc.sync.dma_start(dst, src)
                else:
                    nc.sync.dma_start(tab_s[w * WIN:(w + 1) * WIN, 0:cont],
                                      row_t[:WIN, 0:cont])

            def allgather(l):
                if not timing_reps:
                    tab_s, tab_f = tabs[l]
                    nc.gpsimd.collective_compute(
                        "AllGather", ALU.bypass,
                        replica_groups=[list(range(C))],
                        ins=[tab_s[:, :]], outs=[tab_f[:, :]])

            def oh_window(l, w, ccol):
                """One batched one-hot build for all of window w's chunks."""
                tw = tws[w]
                eng = (nc.gpsimd if os.environ.get("GAT_OH", "dve") == "pool"
                       else nc.vector)
                oh_all = sp.tile([P, tw * WIN], F16, tag="ohall", bufs=2)
                if 'nooh' in abl:
                    nc.vector.memset(oh_all[0:1, 0:2], 0.0)
                else:
                    eng.tensor_tensor(
                        oh_all[:].rearrange("p (t x) -> p t x", x=WIN),
                        iorep[:, 0:tw * WIN].rearrange(
                            "p (t x) -> p t x", x=WIN),
                        dl16_sb[:, ccol:ccol + tw].rearrange(
                            "p (t o) -> p t o", o=1).to_broadcast(
                                [P, tw, WIN]),
                        op=ALU.is_equal)
                return oh_all

            abl = set(os.environ.get("GAT_ABL", "").split(","))

            def edge_front(l, w, icol, ccol, ecol):
                """gather + pads + onehot + scores + weighting for window w."""
                cfg = LAYERS[l]
                heads, scat, row = cfg['heads'], cfg['scat'], cfg['row']
                asf = cfg['asf']
                tab_f = tabs[l][1]
                kw, tw = kws[w], tws[w]
                gw = gp.tile([P, tw * row], F16, tag="gw")
                gw3 = gw[:].rearrange("p (t e) -> p t e", e=row)
                if 'nogather' not in abl:
                    nc.gpsimd.dma_gather(
                        gw3, tab_f[:, :], idx_sb[:, icol:icol + kw // 16],
                        kw, kw, row, single_packet=False,
                        queue_num=w % 4)
                else:
                    nc.vector.memset(gw[0:1, 0:2], 0.0)
                oh_all = oh_window(l, w, ccol)
                ad_w = ado[l][:WIN, w * heads:(w + 1) * heads]
                pads = pp.tile([P, tw * heads], F32, tag="pads", bufs=2)
                if 'nope' not in abl:
                    for t in range(tw):
                        nc.tensor.matmul(
                            pads[:, t * heads:(t + 1) * heads],
                            lhsT=oht_sb[:, ecol + t * P:ecol + (t + 1) * P],
                            rhs=ad_w, start=True, stop=True)
                else:
                    nc.vector.memset(pads[0:1, 0:2], 0.0)
                gwf = gw[:].bitcast(F32).rearrange("p (t e) -> p t e",
                                                   e=row // 2)
                st = sp.tile([P, tw * heads], F32, tag="st")
                lt = sp.tile([P, tw * heads], F32, tag="lt")
                if 'nost' not in abl:
                    nc.vector.tensor_tensor(
                        st[:].rearrange("p (t h) -> p t h", h=heads),
                        gwf[:, :, asf:asf + heads], pads[:].rearrange(
                            "p (t h) -> p t h", h=heads), op=ALU.add)
                    nc.vector.scalar_tensor_tensor(lt[:], st[:], NEG, st[:],
                                                   op0=ALU.mult, op1=ALU.max)
                else:
                    nc.vector.memset(st[0:1, 0:2], 0.0)
                    nc.vector.memset(lt[0:1, 0:2], 0.0)
                p_rep = rp.tile([P, tw * scat], F16, tag="prep")
                lt_b = lt[:].rearrange("p (t h) -> p t h", h=heads)
                if 'noexp' not in abl:
                    nc.scalar.activation(
                        p_rep[:].rearrange("p (t h x) -> p t h x",
                                           h=heads, x=65),
                        lt_b.broadcast_to((P, tw, heads, 65)), ACTF.Exp)
                else:
                    nc.vector.memset(p_rep[0:1, 0:2], 0.0)
                rhs_all = rp.tile([P, tw * scat], F16, tag="rhs")
                if 'noweight' not in abl:
                    nc.vector.tensor_tensor(
                        rhs_all[:].rearrange("p (t e) -> p t e", e=scat),
                        gw3[:, :, 0:scat], p_rep[:].rearrange(
                            "p (t e) -> p t e", e=scat), op=ALU.mult)
                else:
                    nc.vector.memset(rhs_all[0:1, 0:2], 0.0)
                return oh_all, rhs_all

            def edge_back(l, w, fr, actT_next):
                """scatter + epilogue for window w."""
                cfg = LAYERS[l]
                heads, scat = cfg['heads'], cfg['scat']
                tw = tws[w]
                oh_all, rhs_all = fr
                psw = pp.tile([WIN, scat], F32, tag="psw", bufs=2)
                if 'nope' not in abl:
                    for t in range(tw):
                        nc.tensor.matmul(
                            psw[:], lhsT=oh_all[:, t * WIN:(t + 1) * WIN],
                            rhs=rhs_all[:, t * scat:(t + 1) * scat],
                            start=(t == 0), stop=(t == tw - 1))
                else:
                    nc.vector.memset(psw[0:1, 0:2], 0.0)
                ps3 = psw[:].rearrange("p (h x) -> p h x", x=65)
                dn = sp.tile([WIN, heads], F32, tag="dn")
                nc.vector.tensor_scalar(dn[:], ps3[:, :, 64:65], 1e-16, None,
                                        op0=ALU.add)
                rec = sp.tile([WIN, heads], F32, tag="rec")
                nc.vector.reciprocal(rec[:], dn[:])
                out_t = op_.tile([WIN, heads * 64], F32, tag="out_t")
                nc.vector.tensor_tensor(
                    out_t[:].rearrange("p (h x) -> p h x", x=64),
                    ps3[:, :, 0:64], rec[:].broadcast_to((WIN, heads, 64)),
                    op=ALU.mult)
                if l < 2:
                    for j in range(2):
                        pt = pp.tile([P, WIN], F32, tag="pt", bufs=2)
                        if 'nope' not in abl:
                            nc.tensor.transpose(pt[:],
                                                out_t[:, j * P:(j + 1) * P],
                                                ident[:WIN, :WIN])
                        else:
                            nc.vector.memset(pt[0:1, 0:2], 0.0)
                        nc.scalar.activation(
                            actT_next[j][:, w * WIN:(w + 1) * WIN], pt[:],
                            ACTF.Relu, bias=bc12[:, 2 * l + j:2 * l + j + 1])
                else:
                    orow = op_.tile([WIN, OUT_CH], F32, tag="orow")
                    nc.vector.tensor_tensor(orow[:], out_t[:], b3r[:WIN, :],
                                            op=ALU.add)
                    nc.sync.dma_start(d_out[w * WIN:(w + 1) * WIN, :],
                                      orow[:])

            def edge_dense_phase(l, actT_next, next_in):
                """edge(l) with a 1-window software-pipeline skew: window
                w+1's front (gather/pads/onehot/scores/weighting) is issued
                before window w's back (scatter/epilogue), so no engine queue
                blocks the next window behind the current window's tail.
                dense(l+1) is interleaved per window so the next layer's
                table+AG is ready as edge(l) drains."""
                offs, icol, ccol, ecol = [], 0, 0, 0
                for w in range(NW):
                    offs.append((icol, ccol, ecol))
                    icol += kws[w] // 16
                    ccol += kws[w] // P
                    ecol += kws[w]
                fr_prev = None
                for w in range(NW):
                    fr = edge_front(l, w, *offs[w])
                    if fr_prev is not None:
                        edge_back(l, w - 1, fr_prev, actT_next)
                        if next_in is not None:
                            dense_window(l + 1, w - 1, next_in)
                    fr_prev = fr
                edge_back(l, NW - 1, fr_prev, actT_next)
                if next_in is not None:
                    dense_window(l + 1, NW - 1, next_in)
                    allgather(l + 1)

            def body():
                stages = int(os.environ.get("GAT_STAGES", "9"))
                for w in range(NW):
                    dense_window(0, w, [xT])
                allgather(0)
                if stages >= 3:
                    edge_dense_phase(0, actT[1], actT[1] if stages >= 4 else None)
                if stages >= 5:
                    edge_dense_phase(1, actT[2], actT[2] if stages >= 6 else None)
                if stages >= 7:
                    edge_dense_phase(2, None, None)
                if stages < 7:
                    z = op_.tile([WIN, OUT_CH], F32, tag="z", name="z")
                    nc.vector.memset(z[:], 0.0)
                    for w in range(NW):
                        nc.sync.dma_start(d_out[w * WIN:(w + 1) * WIN, :], z[:])

            if timing_reps:
                tk = cp.tile([1, 32], F32)
                nc.sync.dma_start(tk[:], d_tok[:, :])
                if timing_reps == 1:
                    body()
                else:
                    with tc.For_i(0, timing_reps, 1):
                        body()
                nc.sync.dma_start(d_toko[:, :], tk[:])
            else:
                body()

    nc.compile()
    return nc


def _host_inputs(x, edge_index, W1, a1s, a1d, b1, W2, a2s, a2d, b2, W3, a3s, a3d, b3):
    kws, idx_all, dl_all, oht_all = _host_prep(edge_index)
    x = np.asarray(x, dtype=np.float32)
    Ws = [np.asarray(W1, np.float32), np.asarray(W2, np.float32),
          np.asarray(W3, np.float32)]
    As = [np.asarray(a1s, np.float32), np.asarray(a2s, np.float32),
          np.asarray(a3s, np.float32)]
    Ad = [np.asarray(a1d, np.float32), np.asarray(a2d, np.float32),
          np.asarray(a3d, np.float32)]
    bs = [np.asarray(b1, np.float32), np.asarray(b2, np.float32),
          np.asarray(b3, np.float32)]
    shared = {}
    for l, cfg in enumerate(LAYERS):
        fin, heads, scat, wcols = (cfg['fin'], cfg['heads'], cfg['scat'],
                                   cfg['wcols'])
        W, a_s, a_d = Ws[l], As[l], Ad[l]
        Wext = np.zeros((fin, wcols), np.float32)
        for k in range(heads):
            Wk = W[:, 64 * k:64 * (k + 1)]
            Wext[:, 65 * k:65 * k + 64] = Wk
            Wext[:, scat + k] = Wk @ a_s[k]
            Wext[:, scat + heads + k] = Wk @ a_d[k]
        shared[f"Wx{l+1}"] = Wext.astype(NPDT)
    bc = np.zeros((P, 4), np.float32)
    bc[:, 0] = bs[0][0:P]
    bc[:, 1] = bs[0][P:2 * P]
    bc[:, 2] = bs[1][0:P]
    bc[:, 3] = bs[1][P:2 * P]
    shared["bc12"] = bc
    shared["b3r"] = np.tile(bs[2].reshape(1, OUT_CH), (P, 1))
    shared["ident"] = np.eye(P, dtype=np.float32)
    twmax = max(k // P for k in kws)
    shared["iota_rep"] = np.tile(
        np.arange(WIN, dtype=np.float32).reshape(1, WIN),
        (P, twmax)).astype(NPDT)
    in_maps = []
    for c in range(C):
        m = dict(shared)
        m["xT_own"] = np.ascontiguousarray(
            x[c * SHARD:(c + 1) * SHARD].T).astype(NPDT)
        m["gat_idx"] = idx_all[c]
        m["dstloc16"] = dl_all[c].astype(NPDT)
        m["ohT"] = oht_all[c]
        in_maps.append(m)
    return kws, in_maps


_CACHE = {}


def kernel(**inputs) -> np.ndarray:
    kws, in_maps = _host_inputs(**inputs)
    if kws not in _CACHE:
        _CACHE[kws] = build(kws)
    nc = _CACHE[kws]
    last = None
    for _attempt in range(2):
        try:
            res = bass_utils.run_bass_kernel_spmd(
                nc, in_maps, core_ids=list(range(C)), trace=False)
            return np.concatenate(
                [res.results[c]["out"] for c in range(C)], axis=0)
        except Exception as e:  # rare transient device-mesh hiccups: retry once
            last = e
    raise last


# revision 38
# speedup vs baseline: 1.1346x; 1.0720x over previous
"""3-layer GAT on trn2, 8 NeuronCores, edge-parallel with dst-range sharding.

Per core c (owning dst nodes [c*2500, (c+1)*2500)), edges bucketed by dst into
20 windows of 125 nodes, padded per-window to a multiple of 128 (window sizes
maxed over cores so the SPMD program is identical everywhere).

v2 design (vs baseline): fp16 gather table with interleaved [h_k|1]xheads
rows (the 1-columns make the scatter matmul emit softmax denominators for
free) and per-edge `as` stored as fp32 inside the fp16 row via a bitcast
view; as/ad folded into the dense matmul via host-precomputed W@a columns;
the transposed one-hot (ohT) host-precomputed and persistent in SBUF; the
per-window score pipeline is batched (one TT add, one fused leaky
scalar_tensor_tensor, one Act Exp that also expands p per-head to 65 cols via
a stride-0 view, one packed fp16 TT for the p-weighting at DVE 2x mode); per
chunk only a 4x-mode fp16 one-hot build on DVE plus two PE matmuls (ad
gather + scatter). Epilogue divides via one broadcast TT; bias+relu run on
the Act engine in transposed layout where bias is per-partition. The
dma_gather row must be a multiple of 256B, hence row padding to 384/128
elems.
"""
import os, sys
for _p in ('/opt/trn_rl_repo', '/root/.axon_site/_ro/trn_rl_repo'):
    if os.path.isdir(_p) and _p not in sys.path:
        sys.path.insert(0, _p)

import numpy as np

import concourse.bacc as bacc
import concourse.tile as tile
from concourse import bass, mybir
from concourse import bass_utils

N = 20000
E = 320000
HID = 64
HEADS = 4
OUT_CH = 64
NEG = 0.2
C = 8
SHARD = N // C          # 2500
WIN = 125               # dst nodes per window
NW = SHARD // WIN       # 20
P = 128

# fin, fout, heads, scat (=65*heads, scatter cols), row (gather row elems,
# 256B-multiple for dma_gather), asf (fp32 col of `as` in the row's f32
# bitcast view), wcols (dense-matmul rhs cols: scat | as | ad), cont (row
# content cols actually written/read; the rest is dma_gather row padding)
LAYERS = [
    dict(fin=64,  fout=256, heads=4, scat=260, row=384, asf=130, wcols=268,
         cont=268),
    dict(fin=256, fout=256, heads=4, scat=260, row=384, asf=130, wcols=268,
         cont=268),
    dict(fin=256, fout=64,  heads=1, scat=65,  row=128, asf=33,  wcols=67,
         cont=68),
]

AX = mybir.AxisListType
ALU = mybir.AluOpType
ACTF = mybir.ActivationFunctionType
F32 = mybir.dt.float32
F16 = mybir.dt.float16
I16 = mybir.dt.int16
NPDT = np.float16


def _dma_gather_raw(nc, out_ap, in_ap, idxs_ap, num_idxs, elem_size,
                    elem_step, queue_num=0):
    """dma_gather with payload < row stride: elem_size need not be a 256B
    multiple (only the encoded stride is in 256B units); mirrors
    BassGpSimd.dma_gather with transpose=False, prepare_only=False."""
    from concourse.ap_utils import ap_is_contiguous
    g = nc.gpsimd
    assert idxs_ap.dtype == mybir.dt.int16
    assert in_ap.dtype == out_ap.dtype
    assert in_ap.space == bass.MemorySpace.DRAM
    assert idxs_ap.space == bass.MemorySpace.SBUF
    assert out_ap.space == bass.MemorySpace.SBUF
    assert ap_is_contiguous(out_ap.ap[1:])
    assert ap_is_contiguous(idxs_ap.ap[1:])
    assert in_ap.ap[-1][1] == out_ap.ap[-1][1] == elem_size
    assert out_ap.ap[0][1] * out_ap.ap[1][1] == ((num_idxs + 127) // 128) * 128
    assert in_ap.ap[0][0] == elem_step
    stride_bytes = elem_step * mybir.dt.size(in_ap.dtype)
    stride_bytes_256 = stride_bytes // 256
    assert stride_bytes % 256 == 0 and stride_bytes_256 < 256
    _in_ap = g.lower_ap_dma(in_ap, for_custom_bir_dma=True)
    inst = g.add_instruction(
        mybir.InstDMAGatherAnt(
            name=nc.get_next_instruction_name(),
            ins=[*_in_ap, g.lower_ap(idxs_ap),
                 g.lower_val_access(g.to_reg(num_idxs))],
            outs=[g.lower_ap(out_ap)],
            transpose=False,
            num_idxs=num_idxs,
            elem_size=elem_size,
            stride_bytes_256=stride_bytes_256,
            gen_mode=0,
            single_packet=False,
            queue_num=queue_num,
            sbuf_tokens_per_rank=0,
            sbuf_free_dim_per_rank=0,
            sbuf_free_dim_pad_per_rank=0,
            sbuf_byte_offset=0,
        )
    )
    return inst


def _host_prep(edge_index):
    """Per-core gather idx / dstloc / transposed-onehot arrays + window sizes."""
    src = np.asarray(edge_index[0], dtype=np.int64)
    dst = np.asarray(edge_index[1], dtype=np.int64)
    per_core = []   # (srcs, dstloc) per (core, window)
    counts = np.zeros((C, NW), dtype=np.int64)
    for c in range(C):
        m = (dst >= c * SHARD) & (dst < (c + 1) * SHARD)
        es, ed = src[m], dst[m] - c * SHARD
        order = np.argsort(ed, kind='stable')
        es, ed = es[order], ed[order]
        w = ed // WIN
        wins = []
        for wi in range(NW):
            sel = w == wi
            ws, wd = es[sel], ed[sel] - wi * WIN
            # sort the window's edges by src so the gather walks HBM in
            # ascending address order (scatter is one-hot-based, so edge
            # order within a window is free)
            o = np.argsort(ws, kind='stable')
            wins.append((ws[o], wd[o]))
            counts[c, wi] = sel.sum()
        per_core.append(wins)
    kws = (np.ceil(counts.max(axis=0) / P).astype(np.int64) * P)
    kws = np.maximum(kws, P)
    tot = int(kws.sum())
    idx_all, dl_all, oht_all = [], [], []
    for c in range(C):
        idx_mat = np.zeros((16, tot // 16), dtype=np.int16)
        dl_mat = np.full((P, tot // P), float(WIN), dtype=np.float32)
        oht = np.zeros((WIN, tot), dtype=NPDT)
        icol = ccol = ecol = 0
        for wi in range(NW):
            kw = int(kws[wi])
            es, dl = per_core[c][wi]
            n = len(es)
            sp = np.zeros(kw, dtype=np.int16)
            dp = np.full(kw, float(WIN), dtype=np.float32)
            sp[:n] = es.astype(np.int16)
            dp[:n] = dl.astype(np.float32)
            idx_mat[:, icol:icol + kw // 16] = sp.reshape(-1, 16).T
            dl_mat[:, ccol:ccol + kw // P] = dp.reshape(-1, P).T
            oht[dl.astype(np.int64), ecol + np.arange(n)] = NPDT(1.0)
            icol += kw // 16
            ccol += kw // P
            ecol += kw
        idx_all.append(np.tile(idx_mat, (8, 1)))
        dl_all.append(dl_mat)
        oht_all.append(oht)
    return tuple(int(k) for k in kws), idx_all, dl_all, oht_all


def build(kws, timing_reps=0):
    """Builds the SPMD bass module. kws: per-window padded edge counts."""
    tot = sum(kws)
    tws = [k // P for k in kws]
    nc = bacc.Bacc("TRN2", target_bir_lowering=False, debug=False,
                   num_devices=C, num_swdge_queues=4)

    # ---- DRAM I/O ----
    d_xT = nc.dram_tensor("xT_own", [HID, SHARD], F16, kind="ExternalInput")
    d_W = [nc.dram_tensor(f"Wx{l+1}", [LAYERS[l]['fin'], LAYERS[l]['wcols']],
                          F16, kind="ExternalInput") for l in range(3)]
    d_bc = nc.dram_tensor("bc12", [P, 4], F32, kind="ExternalInput")
    d_b3 = nc.dram_tensor("b3r", [P, OUT_CH], F32, kind="ExternalInput")
    d_ident = nc.dram_tensor("ident", [P, P], F32, kind="ExternalInput")
    d_idx = nc.dram_tensor("gat_idx", [P, tot // 16], I16, kind="ExternalInput")
    d_dl16 = nc.dram_tensor("dstloc16", [P, tot // P], F16,
                            kind="ExternalInput")
    twmax = max(tws)
    d_iorep = nc.dram_tensor("iota_rep", [P, twmax * WIN], F16,
                             kind="ExternalInput")
    d_oht = nc.dram_tensor("ohT", [WIN, tot], F16, kind="ExternalInput")
    d_out = nc.dram_tensor("out", [SHARD, OUT_CH], F32, kind="ExternalOutput")
    if timing_reps:
        d_tok = nc.dram_tensor("tok", [1, 32], F32, kind="ExternalInput")
        d_toko = nc.dram_tensor("tok_out", [1, 32], F32, kind="ExternalOutput")

    tabs = []
    for l, cfg in enumerate(LAYERS):
        s = nc.dram_tensor(f"tab{l+1}s", [SHARD, cfg['row']], F16)
        f = nc.dram_tensor(f"tab{l+1}f", [N, cfg['row']], F16,
                           addr_space="Shared")
        tabs.append((s, f))

    with tile.TileContext(nc) as tc:
        with tc.tile_pool(name="const", bufs=1) as cp, \
             tc.tile_pool(name="rowp", bufs=2) as rowp, \
             tc.tile_pool(name="gp", bufs=2) as gp, \
             tc.tile_pool(name="sp", bufs=3) as sp, \
             tc.tile_pool(name="rp", bufs=2) as rp, \
             tc.tile_pool(name="op", bufs=2) as op_, \
             tc.tile_pool(name="ps", bufs=1, space="PSUM") as pp:

            # ---- persistent SBUF ----
            ident = cp.tile([P, P], F32)
            nc.sync.dma_start(ident[:], d_ident[:, :])
            idx_sb = cp.tile([P, tot // 16], I16)
            nc.sync.dma_start(idx_sb[:], d_idx[:, :])
            dl16_sb = cp.tile([P, tot // P], F16)
            nc.sync.dma_start(dl16_sb[:], d_dl16[:, :])
            iorep = cp.tile([P, twmax * WIN], F16)
            nc.sync.dma_start(iorep[:], d_iorep[:, :])
            oht_sb = cp.tile([WIN, tot], F16)
            nc.sync.dma_start(oht_sb[:], d_oht[:, :])
            xT = cp.tile([HID, SHARD], F16)
            nc.sync.dma_start(xT[:], d_xT[:, :])
            bc12 = cp.tile([P, 4], F32)
            nc.sync.dma_start(bc12[:], d_bc[:, :])
            b3r = cp.tile([P, OUT_CH], F32)
            nc.sync.dma_start(b3r[:], d_b3[:, :])
            Wt = []
            for l, cfg in enumerate(LAYERS):
                fin, wcols = cfg['fin'], cfg['wcols']
                chunks = []
                for kc in range(0, fin, P):
                    ke = min(kc + P, fin)
                    t = cp.tile([ke - kc, wcols], F16, tag=f"W{l}_{kc}")
                    nc.sync.dma_start(t[:], d_W[l][kc:ke, :])
                    chunks.append(t)
                Wt.append(chunks)
            actT = {1: [cp.tile([P, SHARD], F16, tag=f"actT1_{j}",
                                name=f"actT1_{j}") for j in range(2)],
                    2: [cp.tile([P, SHARD], F16, tag=f"actT2_{j}",
                                name=f"actT2_{j}") for j in range(2)]}
            ado = [cp.tile([P, NW * LAYERS[l]['heads']], F16, tag=f"ado{l}",
                           name=f"ado{l}") for l in range(3)]

            def dense_window(l, w, actT_in):
                cfg = LAYERS[l]
                heads, scat, row, wcols = (cfg['heads'], cfg['scat'],
                                           cfg['row'], cfg['wcols'])
                asf = cfg['asf']
                tab_s, tab_f = tabs[l]
                nchunks = len(Wt[l])
                ph = pp.tile([WIN, wcols], F32, tag="ph", bufs=2)
                for kc in range(nchunks):
                    nc.tensor.matmul(
                        ph[:, :], lhsT=actT_in[kc][:, w * WIN:(w + 1) * WIN],
                        rhs=Wt[l][kc][:], start=(kc == 0),
                        stop=(kc == nchunks - 1))
                row_t = rowp.tile([P, row], F16, tag="row")
                nc.vector.tensor_copy(row_t[:WIN, 0:scat], ph[:, 0:scat])
                ones_v = row_t[:WIN, 0:scat].rearrange(
                    "p (h x) -> p h x", x=65)[:, :, 64:65]
                nc.vector.memset(ones_v, 1.0)
                row_f = row_t[:].bitcast(F32)
                nc.vector.tensor_copy(row_f[:WIN, asf:asf + heads],
                                      ph[:, scat:scat + heads])
                nc.vector.tensor_copy(ado[l][:WIN, w * heads:(w + 1) * heads],
                                      ph[:, scat + heads:scat + 2 * heads])
                cont = cfg['cont']
                if timing_reps:
                    dst = tab_f[:, :].rearrange("(s n) e -> n s e", s=C)[
                        w * WIN:(w + 1) * WIN, :, 0:cont]
                    src = row_t[:WIN, 0:cont].rearrange(
                        "p (o e) -> p o e", o=1).to_broadcast([WIN, C, cont])
                    nc.sync.dma_start(dst, src)
                else:
                    nc.sync.dma_start(tab_s[w * WIN:(w + 1) * WIN, 0:cont],
                                      row_t[:WIN, 0:cont])

            def allgather(l):
                if not timing_reps:
                    tab_s, tab_f = tabs[l]
                    nc.gpsimd.collective_compute(
                        "AllGather", ALU.bypass,
                        replica_groups=[list(range(C))],
                        ins=[tab_s[:, :]], outs=[tab_f[:, :]])

            def oh_window(l, w, ccol):
                """One batched one-hot build for all of window w's chunks."""
                tw = tws[w]
                eng = (nc.gpsimd if os.environ.get("GAT_OH", "dve") == "pool"
                       else nc.vector)
                oh_all = sp.tile([P, tw * WIN], F16, tag="ohall", bufs=2)
                if 'nooh' in abl:
                    nc.vector.memset(oh_all[0:1, 0:2], 0.0)
                else:
                    eng.tensor_tensor(
                        oh_all[:].rearrange("p (t x) -> p t x", x=WIN),
                        iorep[:, 0:tw * WIN].rearrange(
                            "p (t x) -> p t x", x=WIN),
                        dl16_sb[:, ccol:ccol + tw].rearrange(
                            "p (t o) -> p t o", o=1).to_broadcast(
                                [P, tw, WIN]),
                        op=ALU.is_equal)
                return oh_all

            abl = set(os.environ.get("GAT_ABL", "").split(","))

            def edge_front(l, w, icol, ccol, ecol):
                """gather + pads + onehot + scores + weighting for window w."""
                cfg = LAYERS[l]
                heads, scat, row = cfg['heads'], cfg['scat'], cfg['row']
                asf = cfg['asf']
                tab_f = tabs[l][1]
                kw, tw = kws[w], tws[w]
                gcols = cfg['cont'] if l < 2 else row
                gw = gp.tile([P, tw * gcols], F16, tag="gw")
                gw3 = gw[:].rearrange("p (t e) -> p t e", e=gcols)
                if 'nogather' not in abl:
                    if gcols == row:
                        nc.gpsimd.dma_gather(
                            gw3, tab_f[:, :], idx_sb[:, icol:icol + kw // 16],
                            kw, kw, row, single_packet=False,
                            queue_num=w % 4)
                    else:
                        _dma_gather_raw(
                            nc, gw3, tab_f[:, 0:gcols],
                            idx_sb[:, icol:icol + kw // 16], kw, gcols, row,
                            queue_num=w % 4)
                else:
                    nc.vector.memset(gw[0:1, 0:2], 0.0)
                oh_all = oh_window(l, w, ccol)
                ad_w = ado[l][:WIN, w * heads:(w + 1) * heads]
                pads = pp.tile([P, tw * heads], F32, tag="pads", bufs=2)
                if 'nope' not in abl:
                    for t in range(tw):
                        nc.tensor.matmul(
                            pads[:, t * heads:(t + 1) * heads],
                            lhsT=oht_sb[:, ecol + t * P:ecol + (t + 1) * P],
                            rhs=ad_w, start=True, stop=True)
                else:
                    nc.vector.memset(pads[0:1, 0:2], 0.0)
                gwf = gw[:].bitcast(F32).rearrange("p (t e) -> p t e",
                                                   e=gcols // 2)
                st = sp.tile([P, tw * heads], F32, tag="st")
                lt = sp.tile([P, tw * heads], F32, tag="lt")
                if 'nost' not in abl:
                    nc.vector.tensor_tensor(
                        st[:].rearrange("p (t h) -> p t h", h=heads),
                        gwf[:, :, asf:asf + heads], pads[:].rearrange(
                            "p (t h) -> p t h", h=heads), op=ALU.add)
                    nc.vector.scalar_tensor_tensor(lt[:], st[:], NEG, st[:],
                                                   op0=ALU.mult, op1=ALU.max)
                else:
                    nc.vector.memset(st[0:1, 0:2], 0.0)
                    nc.vector.memset(lt[0:1, 0:2], 0.0)
                p_rep = rp.tile([P, tw * scat], F16, tag="prep")
                lt_b = lt[:].rearrange("p (t h) -> p t h", h=heads)
                if 'noexp' not in abl:
                    nc.scalar.activation(
                        p_rep[:].rearrange("p (t h x) -> p t h x",
                                           h=heads, x=65),
                        lt_b.broadcast_to((P, tw, heads, 65)), ACTF.Exp)
                else:
                    nc.vector.memset(p_rep[0:1, 0:2], 0.0)
                rhs_all = rp.tile([P, tw * scat], F16, tag="rhs")
                if 'noweight' not in abl:
                    nc.vector.tensor_tensor(
                        rhs_all[:].rearrange("p (t e) -> p t e", e=scat),
                        gw3[:, :, 0:scat], p_rep[:].rearrange(
                            "p (t e) -> p t e", e=scat), op=ALU.mult)
                else:
                    nc.vector.memset(rhs_all[0:1, 0:2], 0.0)
                return oh_all, rhs_all

            def edge_back(l, w, fr, actT_next):
                """scatter + epilogue for window w."""
                cfg = LAYERS[l]
                heads, scat = cfg['heads'], cfg['scat']
                tw = tws[w]
                oh_all, rhs_all = fr
                psw = pp.tile([WIN, scat], F32, tag="psw", bufs=2)
                if 'nope' not in abl:
                    for t in range(tw):
                        nc.tensor.matmul(
                            psw[:], lhsT=oh_all[:, t * WIN:(t + 1) * WIN],
                            rhs=rhs_all[:, t * scat:(t + 1) * scat],
                            start=(t == 0), stop=(t == tw - 1))
                else:
                    nc.vector.memset(psw[0:1, 0:2], 0.0)
                ps3 = psw[:].rearrange("p (h x) -> p h x", x=65)
                dn = sp.tile([WIN, heads], F32, tag="dn")
                nc.vector.tensor_scalar(dn[:], ps3[:, :, 64:65], 1e-16, None,
                                        op0=ALU.add)
                rec = sp.tile([WIN, heads], F32, tag="rec")
                nc.vector.reciprocal(rec[:], dn[:])
                out_t = op_.tile([WIN, heads * 64], F32, tag="out_t")
                nc.vector.tensor_tensor(
                    out_t[:].rearrange("p (h x) -> p h x", x=64),
                    ps3[:, :, 0:64], rec[:].broadcast_to((WIN, heads, 64)),
                    op=ALU.mult)
                if l < 2:
                    for j in range(2):
                        pt = pp.tile([P, WIN], F32, tag="pt", bufs=2)
                        if 'nope' not in abl:
                            nc.tensor.transpose(pt[:],
                                                out_t[:, j * P:(j + 1) * P],
                                                ident[:WIN, :WIN])
                        else:
                            nc.vector.memset(pt[0:1, 0:2], 0.0)
                        nc.scalar.activation(
                            actT_next[j][:, w * WIN:(w + 1) * WIN], pt[:],
                            ACTF.Relu, bias=bc12[:, 2 * l + j:2 * l + j + 1])
                else:
                    orow = op_.tile([WIN, OUT_CH], F32, tag="orow")
                    nc.vector.tensor_tensor(orow[:], out_t[:], b3r[:WIN, :],
                                            op=ALU.add)
                    nc.sync.dma_start(d_out[w * WIN:(w + 1) * WIN, :],
                                      orow[:])

            def edge_dense_phase(l, actT_next, next_in):
                """edge(l) with a 1-window software-pipeline skew: window
                w+1's front (gather/pads/onehot/scores/weighting) is issued
                before window w's back (scatter/epilogue), so no engine queue
                blocks the next window behind the current window's tail.
                dense(l+1) is interleaved per window so the next layer's
                table+AG is ready as edge(l) drains."""
                offs, icol, ccol, ecol = [], 0, 0, 0
                for w in range(NW):
                    offs.append((icol, ccol, ecol))
                    icol += kws[w] // 16
                    ccol += kws[w] // P
                    ecol += kws[w]
                fr_prev = None
                for w in range(NW):
                    fr = edge_front(l, w, *offs[w])
                    if fr_prev is not None:
                        edge_back(l, w - 1, fr_prev, actT_next)
                        if next_in is not None:
                            dense_window(l + 1, w - 1, next_in)
                    fr_prev = fr
                edge_back(l, NW - 1, fr_prev, actT_next)
                if next_in is not None:
                    dense_window(l + 1, NW - 1, next_in)
                    allgather(l + 1)

            def body():
                stages = int(os.environ.get("GAT_STAGES", "9"))
                for w in range(NW):
                    dense_window(0, w, [xT])
                allgather(0)
                if stages >= 3:
                    edge_dense_phase(0, actT[1], actT[1] if stages >= 4 else None)
                if stages >= 5:
                    edge_dense_phase(1, actT[2], actT[2] if stages >= 6 else None)
                if stages >= 7:
                    edge_dense_phase(2, None, None)
                if stages < 7:
                    z = op_.tile([WIN, OUT_CH], F32, tag="z", name="z")
                    nc.vector.memset(z[:], 0.0)
                    for w in range(NW):
                        nc.sync.dma_start(d_out[w * WIN:(w + 1) * WIN, :], z[:])

            if timing_reps:
                tk = cp.tile([1, 32], F32)
                nc.sync.dma_start(tk[:], d_tok[:, :])
                if timing_reps == 1:
                    body()
                else:
                    with tc.For_i(0, timing_reps, 1):
                        body()
                nc.sync.dma_start(d_toko[:, :], tk[:])
            else:
                body()

    nc.compile()
    return nc


def _host_inputs(x, edge_index, W1, a1s, a1d, b1, W2, a2s, a2d, b2, W3, a3s, a3d, b3):
    kws, idx_all, dl_all, oht_all = _host_prep(edge_index)
    x = np.asarray(x, dtype=np.float32)
    Ws = [np.asarray(W1, np.float32), np.asarray(W2, np.float32),
          np.asarray(W3, np.float32)]
    As = [np.asarray(a1s, np.float32), np.asarray(a2s, np.float32),
          np.asarray(a3s, np.float32)]
    Ad = [np.asarray(a1d, np.float32), np.asarray(a2d, np.float32),
          np.asarray(a3d, np.float32)]
    bs = [np.asarray(b1, np.float32), np.asarray(b2, np.float32),
          np.asarray(b3, np.float32)]
    shared = {}
    for l, cfg in enumerate(LAYERS):
        fin, heads, scat, wcols = (cfg['fin'], cfg['heads'], cfg['scat'],
                                   cfg['wcols'])
        W, a_s, a_d = Ws[l], As[l], Ad[l]
        Wext = np.zeros((fin, wcols), np.float32)
        for k in range(heads):
            Wk = W[:, 64 * k:64 * (k + 1)]
            Wext[:, 65 * k:65 * k + 64] = Wk
            Wext[:, scat + k] = Wk @ a_s[k]
            Wext[:, scat + heads + k] = Wk @ a_d[k]
        shared[f"Wx{l+1}"] = Wext.astype(NPDT)
    bc = np.zeros((P, 4), np.float32)
    bc[:, 0] = bs[0][0:P]
    bc[:, 1] = bs[0][P:2 * P]
    bc[:, 2] = bs[1][0:P]
    bc[:, 3] = bs[1][P:2 * P]
    shared["bc12"] = bc
    shared["b3r"] = np.tile(bs[2].reshape(1, OUT_CH), (P, 1))
    shared["ident"] = np.eye(P, dtype=np.float32)
    twmax = max(k // P for k in kws)
    shared["iota_rep"] = np.tile(
        np.arange(WIN, dtype=np.float32).reshape(1, WIN),
        (P, twmax)).astype(NPDT)
    in_maps = []
    for c in range(C):
        m = dict(shared)
        m["xT_own"] = np.ascontiguousarray(
            x[c * SHARD:(c + 1) * SHARD].T).astype(NPDT)
        m["gat_idx"] = idx_all[c]
        m["dstloc16"] = dl_all[c].astype(NPDT)
        m["ohT"] = oht_all[c]
        in_maps.append(m)
    return kws, in_maps


_CACHE = {}


def kernel(**inputs) -> np.ndarray:
    kws, in_maps = _host_inputs(**inputs)
    if kws not in _CACHE:
        _CACHE[kws] = build(kws)
    nc = _CACHE[kws]
    last = None
    for _attempt in range(2):
        try:
            res = bass_utils.run_bass_kernel_spmd(
                nc, in_maps, core_ids=list(range(C)), trace=False)
            return np.concatenate(
                [res.results[c]["out"] for c in range(C)], axis=0)
        except Exception as e:  # rare transient device-mesh hiccups: retry once
            last = e
    raise last
